# revision 52
# baseline (speedup 1.0000x reference)
"""Multi-head attention (B=2, S=2048, D=1024, H=16) on 8 Trainium2 cores.

Sharding: core c handles batch b = c//4 and head group g = c%4 (4 heads,
256 of the 1024 QKV output columns). Zero-communication: each core
computes its output slice fully.

Per-core structure (v2 — scheduled for ACT-bound steady state):

  1. Inputs DMA'd in chunks (xT by 512-col chunks, keepT by j-blocks) so
     projections can start ~5us in.
  2. q/k projected in transposed layout qT/kT [dh, s] in 512-col chunks
     (8 K-tiled matmuls each, bias folded into the DVE eviction as a
     per-partition tensor_scalar_add). v projected in natural layout
     [s, dh] per 128-row tile, all 4 heads at once (xT tile stationary),
     with the bias as a rank-1 ones*bv matmul into the same PSUM group.
     Only the first chunks are emitted up front; the rest ride as
     fillers inside the attention loop's PE slack (attention is
     ACT-bound at ~1.1us/iter vs ~0.64us of PE work).
  3. Attention per head-pair in transposed layout: logitsT[sk, sq] =
     kT_tile.T @ qT (two K=64 matmuls row-packed on disjoint PE row
     halves, concurrent), expw = Exp(scale*logits) on ACT, masked by
     multiplying with keepT = (~mask).T in bf16 {0,1} on DVE (exact:
     exp(-1e9) underflows to 0 in fp32).
  4. PV with a ones-augmented V: out_augT[dh+1, sq] += [v|1].T @ expw —
     row 64 accumulates the softmax denominator for free.
  5. Tail per (head, j-block): evict PV PSUM -> bf16 SBUF, transpose
     back to natural [sq, dh] with the DMA xbar (dma_start_transpose,
     frees the PE and most of the old DVE tail), normalize rows by
     1/rowsum on DVE, DMA out.

Matmuls run in bf16 (inputs cast on host), accumulation in fp32 PSUM.
"""

import numpy as np

B, S, D, H = 2, 2048, 1024, 16
HD = D // H  # 64
HEADS_PER_CORE = 4
COLS = HEADS_PER_CORE * HD  # 256
N_CORES = 8
KT = D // 128  # 8 contraction tiles for projections
ST = S // 128  # 16 s tiles
NC = 4  # 512-col chunks of S
SCALE = 1.0 / np.sqrt(np.float32(D))

_cache = {}


def _build_nc():
    import concourse.bass as bass
    import concourse.mybir as mybir
    import concourse.tile as tile

    f32 = mybir.dt.float32
    bf16 = mybir.dt.bfloat16

    nc = bass.Bass(trn_type="TRN2")

    f8 = mybir.dt.float8e4

    xT = nc.dram_tensor("xT", [D, S], bf16, kind="ExternalInput")
    wq = nc.dram_tensor("wq", [D, COLS], bf16, kind="ExternalInput")
    wk = nc.dram_tensor("wk", [D, COLS], bf16, kind="ExternalInput")
    wv = nc.dram_tensor("wv", [D, COLS], bf16, kind="ExternalInput")
    bq = nc.dram_tensor("bq", [128, 2], f32, kind="ExternalInput")
    bk = nc.dram_tensor("bk", [128, 2], f32, kind="ExternalInput")
    bv = nc.dram_tensor("bv", [128, 2], f32, kind="ExternalInput")
    keepT = nc.dram_tensor("keepT", [S, S], bf16, kind="ExternalInput")
    o = nc.dram_tensor("o", [S, COLS], f32, kind="ExternalOutput")

    with tile.TileContext(nc) as tc:
        with (
            tc.tile_pool(name="singles", bufs=1) as singles,
            tc.tile_pool(name="persist", bufs=1) as persist,
            tc.tile_pool(name="big_ps", bufs=2, space="PSUM") as big_ps,
            tc.tile_pool(name="pv_ps", bufs=2, space="PSUM") as pv_ps,
            tc.tile_pool(name="tr_ps", bufs=2, space="PSUM") as tr_ps,
            tc.tile_pool(name="expw", bufs=8) as expw_pool,
            tc.tile_pool(name="expw2", bufs=8) as expw2_pool,
            tc.tile_pool(name="tails", bufs=6) as tails,
            tc.tile_pool(name="vstage", bufs=2) as vstage,
        ):
            # ---- constants / small inputs ----
            ones_col = singles.tile([1, 128], bf16)
            nc.vector.memset(ones_col, 1.0)

            # ---- bulk inputs. HBM is ~0.35 MB/us per core and the total
            # input is ~17 MB (~50us), so ordering decides when compute can
            # start: the first QK needs only wk blk0 + xT chunk0 + wq blk0
            # (1.5 MB), and the first mask-mul needs keepT j0 (2 MB, on the
            # otherwise-idle ACT hwdge queue so its issue doesn't serialize
            # behind the Sync queue).
            wk_sb = persist.tile([128, KT, COLS], bf16)
            wq_sb = persist.tile([128, KT, COLS], bf16)
            wv_sb = persist.tile([128, KT, COLS], bf16)
            xT_sb = persist.tile([128, KT, S], bf16)
            keepT_sb = persist.tile([128, ST, S], bf16)
            bq_sb = singles.tile([128, 2], f32)
            bk_sb = singles.tile([128, 2], f32)
            bv_sb = singles.tile([128, 2], f32)

            xT_r = xT[:, :].rearrange("(kt p) s -> p kt s", p=128)
            keepT_r = keepT[:, :].rearrange("(i p) s -> p i s", p=128)

            def xchunk(c, eng):
                eng.dma_start(
                    out=xT_sb[:, :, c * 512 : (c + 1) * 512],
                    in_=xT_r[:, :, c * 512 : (c + 1) * 512],
                )

            def kchunk(j, eng):
                eng.dma_start(
                    out=keepT_sb[:, :, j * 512 : (j + 1) * 512],
                    in_=keepT_r[:, :, j * 512 : (j + 1) * 512],
                )

            def wblk(w_sb, w_dram, blk, eng):
                eng.dma_start(
                    out=w_sb[:, :, blk * 128 : (blk + 1) * 128],
                    in_=w_dram[:, blk * 128 : (blk + 1) * 128].rearrange(
                        "(kt p) c -> p kt c", p=128
                    ),
                )

            kchunk(0, nc.scalar)  # ACT hwdge queue — idle until the first exp
            # Pre-load the ACT exp table set during the DMA window so the
            # first real ACTIVATE doesn't pay the ~2.7us table load.
            scratch = singles.tile([1, 128], bf16)
            nc.scalar.activation(
                out=scratch,
                in_=ones_col,
                func=mybir.ActivationFunctionType.Exp,
            )
            nc.scalar.dma_start(
                out=wv_sb, in_=wv[:, :].rearrange("(kt p) c -> p kt c", p=128)
            )
            xchunk(2, nc.scalar)
            xchunk(3, nc.scalar)
            nc.sync.dma_start(
                out=wk_sb, in_=wk[:, :].rearrange("(kt p) c -> p kt c", p=128)
            )
            xchunk(0, nc.sync)
            nc.sync.dma_start(
                out=wq_sb, in_=wq[:, :].rearrange("(kt p) c -> p kt c", p=128)
            )
            nc.sync.dma_start(out=bk_sb, in_=bk[:, :])
            nc.sync.dma_start(out=bq_sb, in_=bq[:, :])
            xchunk(1, nc.sync)
            nc.sync.dma_start(out=bv_sb, in_=bv[:, :])
            kchunk(1, nc.sync)
            kchunk(2, nc.sync)
            kchunk(3, nc.sync)

            # ---- projection targets ----
            qT_sb = persist.tile([128, 2, S], bf16)
            kT_sb = persist.tile([128, 2, S], bf16)
            # v is projected in the same transposed layout vT [2 heads x 64,
            # s] (weights stationary: 1.7us per 512-col chunk, vs ~4us for
            # the x-stationary natural-layout form whose per-(kt,st)
            # LDWEIGHTS dominates), then DMA-xbar-transposed per 128-col
            # tile into the natural ones-augmented layout the PV matmul
            # needs: v_aug[p, st, h, 0:64] = v, v_aug[p, st, h, 64] = 1.
            vT_sb = persist.tile([128, 2, S], bf16)
            v_aug = persist.tile([128, ST, HEADS_PER_CORE, HD + 1], bf16)
            nc.vector.memset(v_aug[:, :, :, HD : HD + 1], 1.0)

            # Projections are emitted as small "quarters" (2 matmuls each) so
            # a filler never delays the QK->ACT critical path by more than
            # ~0.45us — one quarter fits the per-iteration PE slack under the
            # ACT-paced steady state. Each chunk is 4 quarters sharing a
            # PSUM tile via closure state; the last quarter evicts.
            def make_qk_quarters(which, blk, c):
                w_sb, b_sb, dst = (
                    (wq_sb, bq_sb, qT_sb),
                    (wk_sb, bk_sb, kT_sb),
                    (wv_sb, bv_sb, vT_sb),
                )[which]
                state = {}

                def quarter(k0):
                    def run():
                        if k0 == 0:
                            state["ps"] = tr_ps.tile(
                                [128, 512], f32, tag="tr", name="qk_ps"
                            )
                        ps = state["ps"]
                        for kt in range(k0, k0 + 2):
                            nc.tensor.matmul(
                                ps,
                                lhsT=w_sb[:, kt, blk * 128 : (blk + 1) * 128],
                                rhs=xT_sb[:, kt, c * 512 : (c + 1) * 512],
                                start=(kt == 0),
                                stop=(kt == KT - 1),
                                skip_group_check=True,
                            )
                        if k0 == 6:
                            nc.vector.tensor_scalar_add(
                                out=dst[:, blk, c * 512 : (c + 1) * 512],
                                in0=ps,
                                scalar1=b_sb[:, blk : blk + 1],
                            )

                    return run

                return [quarter(k0) for k0 in (0, 2, 4, 6)]

            # After a vT chunk is projected, one [128, 512] xbar transpose
            # moves it to natural layout in a contiguous staging tile
            # (3D out [128, 4, 128]: transposed row r = c*128 + p = s
            # within the chunk), then a strided DVE copy interleaves it
            # into v_aug's per-head 65-col groups.
            def make_v_quarters(p, c):
                qs = make_qk_quarters(2, p, c)
                q3 = qs[3]

                def last():
                    q3()
                    vst = vstage.tile([128, 4, 2 * HD], bf16, name="vst")
                    nc.sync.dma_start_transpose(
                        out=vst,
                        in_=vT_sb[:, p, c * 512 : (c + 1) * 512],
                    )
                    nc.vector.tensor_copy(
                        out=v_aug[:, 4 * c : 4 * c + 4, 2 * p : 2 * p + 2, 0:HD],
                        in_=vst.rearrange("p c (h d) -> p c h d", h=2),
                    )

                return [qs[0], qs[1], qs[2], last]

            # ---- attention ----
            # Head pair (2hp, 2hp+1) shares one [128,1024] logits PSUM tile:
            # head e in cols 0-511 (PE rows 0-63), head o in cols 512-1023
            # (PE rows 64-127 via auto tile_position). The two K=64 matmuls
            # run concurrently on disjoint row groups; one wide exp covers
            # both heads.
            def attention_pair(hp, fillers):
                blk = hp
                for j in range(4):  # sq blocks of 512
                    pvs = [
                        pv_ps.tile([HD + 1, 512], f32, tag="pv", name=f"pv{e}")
                        for e in range(2)
                    ]
                    pending_pv = None
                    for i in range(ST):  # sk tiles of 128
                        lgp = big_ps.tile([128, 1024], f32, tag="big")
                        for e in range(2):
                            po = e * 64
                            nc.tensor.matmul(
                                lgp[:, e * 512 : (e + 1) * 512],
                                lhsT=kT_sb[
                                    po : po + 64, blk, i * 128 : (i + 1) * 128
                                ],
                                rhs=qT_sb[
                                    po : po + 64, blk, j * 512 : (j + 1) * 512
                                ],
                                start=True,
                                stop=True,
                                skip_group_check=True,
                            )
                        # PV of the previous iteration is emitted AFTER this
                        # iteration's QK: its mask dependency resolved during
                        # the previous exp, so it never stalls the PE queue
                        # ahead of the QK the ACT critical path waits on.
                        if pending_pv is not None:
                            pending_pv()
                        for f in fillers.get((j, i), ()):
                            f()
                        ex = expw_pool.tile([128, 1024], bf16)
                        nc.scalar.activation(
                            out=ex,
                            in_=lgp,
                            func=mybir.ActivationFunctionType.Exp,
                            scale=float(SCALE),
                        )
                        # mask: multiply both heads' halves by the same keepT
                        # slice, read twice via a stride-0 broadcast dim
                        ex2 = expw2_pool.tile([128, 1024], bf16)
                        k_ap = keepT_sb[:, i, j * 512 : (j + 1) * 512]
                        k_bcast = bass.AP(
                            tensor=k_ap.tensor,
                            offset=k_ap.offset,
                            ap=[k_ap.ap[0], [0, 2], *k_ap.ap[1:]],
                        )
                        nc.vector.tensor_mul(
                            out=ex2.rearrange("p (e n) -> p e n", e=2),
                            in0=ex.rearrange("p (e n) -> p e n", e=2),
                            in1=k_bcast,
                        )

                        def mk_pv(i, ex2):
                            def run():
                                for e in range(2):
                                    nc.tensor.matmul(
                                        pvs[e],
                                        lhsT=v_aug[:, i, 2 * hp + e, :],
                                        rhs=ex2[:, e * 512 : (e + 1) * 512],
                                        start=(i == 0),
                                        stop=(i == ST - 1),
                                        skip_group_check=True,
                                    )

                            return run

                        pending_pv = mk_pv(i, ex2)
                    pending_pv()
                    # tail: evict both heads to bf16 SBUF first (frees pv
                    # slots), then DMA-xbar-transpose to natural layout,
                    # normalize by the rowsum, store.
                    pv_sbs = []
                    for e in range(2):
                        pv_sb = tails.tile(
                            [80, 512], bf16, tag="pvsb", name=f"pv_sb{e}"
                        )
                        nc.vector.tensor_copy(out=pv_sb[0 : HD + 1, :], in_=pvs[e])
                        pv_sbs.append(pv_sb)
                    for e in range(2):
                        h = 2 * hp + e
                        nat = tails.tile([128, 4, 80], bf16, tag="nat")
                        nc.sync.dma_start_transpose(out=nat, in_=pv_sbs[e][:, :])
                        rc = tails.tile([128, 4], f32, tag="rc")
                        nc.vector.reciprocal(
                            out=rc,
                            in_=nat[:, :, HD : HD + 1].rearrange(
                                "p c one -> p (c one)"
                            ),
                        )
                        ob = tails.tile([128, 4, HD], f32, tag="ob")
                        for c in range(4):
                            nc.vector.tensor_scalar_mul(
                                out=ob[:, c, :],
                                in0=nat[:, c, 0:HD],
                                scalar1=rc[:, c : c + 1],
                            )
                        nc.sync.dma_start(
                            out=o[
                                j * 512 : (j + 1) * 512, h * HD : (h + 1) * HD
                            ].rearrange("(c p) d -> p c d", p=128),
                            in_=ob,
                        )

            # ---- emission order = scheduling priority ----
            Q, K = 0, 1
            # ~16 dummy matmuls on memset data, accumulating into a pv-pool
            # PSUM tile (unused until the first PV at ~26us): the PE's HAM
            # clock gate only reaches 2.4 GHz after ~3.4us of sustained
            # activity and the prologue's DMAs land around t=20us — these
            # warm the array during the dead window so the prologue and
            # early fillers run at full clock.
            warm_sb = singles.tile([128, 512], bf16)
            nc.vector.memset(warm_sb, 0.0)
            warm_ps = pv_ps.tile([HD + 1, 512], f32, tag="pv", name="warm_ps")
            NWARM = 16
            for w in range(NWARM):
                nc.tensor.matmul(
                    warm_ps,
                    lhsT=warm_sb[:, 0 : HD + 1],
                    rhs=warm_sb[:, :],
                    start=(w == 0),
                    stop=(w == NWARM - 1),
                    skip_group_check=True,
                )
            # prologue: only what iteration (0,0) needs — k/q blk0 cols
            # 0-511 and v pair0 chunk0. k and q groups interleaved so each
            # group's DVE eviction overlaps the other group's matmuls.
            k0c0 = make_qk_quarters(K, 0, 0)
            q0c0 = make_qk_quarters(Q, 0, 0)
            for a, b in zip(k0c0, q0c0):
                a()
                b()
            for u in make_v_quarters(0, 0):
                u()

            qk_units = {
                (w, blk, c): make_qk_quarters(w, blk, c)
                for w in (K, Q)
                for blk in (0, 1)
                for c in range(NC)
                if not (blk == 0 and c == 0)
            }
            v_units = {
                (p, c): make_v_quarters(p, c)
                for p in range(2)
                for c in range(NC)
                if not (p == 0 and c == 0)
            }

            # Quarters placed by deadline, at most one (2 matmuls ≈ the PE
            # slack under an ACT-paced iteration) per iteration wherever
            # possible. Hard deadlines: K0/V0 chunk c feed j-block 0 from
            # iteration 4c; Q0 chunk c feeds j-block c; pair-1 units (K1,
            # V1, Q1) are needed from pair-1 iteration 4c / j-block c.
            def place(fillers, j, i0, us):
                for n, u in enumerate(us):
                    idx = i0 + n
                    fillers.setdefault((j + idx // ST, idx % ST), []).append(u)

            fillers0, fillers1 = {}, {}
            # j0: K0/V0 chunks 1-3 must land just ahead of their i-tiles —
            # two quarters per iteration here (PE-heavy window, unavoidable).
            place(fillers0, 0, 0, qk_units[(K, 0, 1)])
            place(fillers0, 0, 0, v_units[(0, 1)])
            place(fillers0, 0, 4, qk_units[(K, 0, 2)])
            place(fillers0, 0, 4, v_units[(0, 2)])
            place(fillers0, 0, 8, qk_units[(K, 0, 3)])
            place(fillers0, 0, 8, v_units[(0, 3)])
            place(fillers0, 0, 12, qk_units[(Q, 0, 1)])
            # j1..j3: one quarter per iteration.
            place(fillers0, 1, 0, qk_units[(K, 1, 0)])
            place(fillers0, 1, 4, v_units[(1, 0)])
            place(fillers0, 1, 8, qk_units[(Q, 0, 2)])
            place(fillers0, 1, 12, qk_units[(K, 1, 1)])
            place(fillers0, 2, 0, v_units[(1, 1)])
            place(fillers0, 2, 4, qk_units[(K, 1, 2)])
            place(fillers0, 2, 8, qk_units[(Q, 0, 3)])
            place(fillers0, 2, 12, v_units[(1, 2)])
            place(fillers0, 3, 0, qk_units[(K, 1, 3)])
            place(fillers0, 3, 4, v_units[(1, 3)])
            place(fillers0, 3, 8, qk_units[(Q, 1, 0)])
            place(fillers1, 0, 0, qk_units[(Q, 1, 1)])
            place(fillers1, 1, 0, qk_units[(Q, 1, 2)])
            place(fillers1, 2, 0, qk_units[(Q, 1, 3)])
            attention_pair(0, fillers0)
            attention_pair(1, fillers1)

    # Workaround: this container's walrus encodes at most one sync wait per
    # instruction — split multi-wait instructions into single-wait NoOps.
    _split_multiwait(nc)
    return nc


def _split_multiwait(nc, max_waits: int = 1):
    import concourse.mybir as mybir

    for f in nc.m.functions:
        for blk in f.blocks:
            out = []
            changed = False
            for inst in blk.instructions:
                si = inst.sync_info
                if si is not None and len(si.on_wait) > max_waits:
                    waits = list(si.on_wait)
                    extra = waits[: len(waits) - max_waits]
                    keep = waits[len(waits) - max_waits :]
                    for k, w in enumerate(extra):
                        out.append(
                            mybir.InstNoOp(
                                name=f"{inst.name}-wfx{k}",
                                engine=inst.engine,
                                sync_info=mybir.SyncInfo(on_wait=[w], on_update=[]),
                                bass_nofuse=True,
                            )
                        )
                    inst.sync_info = mybir.SyncInfo(
                        on_wait=keep, on_update=list(si.on_update)
                    )
                    changed = True
                out.append(inst)
            if changed:
                blk.instructions = out


def _prep_in_maps(x, mask, Wq, bq, Wk, bk, Wv, bv):
    import ml_dtypes

    bf16 = ml_dtypes.bfloat16
    x = np.asarray(x, np.float32)
    mask = np.asarray(mask, bool)

    xT_b = [np.ascontiguousarray(x[b].T).astype(bf16) for b in range(B)]
    keepT_b = [
        np.ascontiguousarray((~mask[b, 0]).T).astype(bf16) for b in range(B)
    ]
    WqT = np.asarray(Wq, np.float32).T.astype(bf16)
    WkT = np.asarray(Wk, np.float32).T.astype(bf16)
    WvT = np.asarray(Wv, np.float32).T.astype(bf16)
    bq32 = np.asarray(bq, np.float32)
    bk32 = np.asarray(bk, np.float32)
    bv32 = np.asarray(bv, np.float32)

    in_maps = []
    for c in range(N_CORES):
        b, g = divmod(c, 4)
        cols = slice(g * COLS, (g + 1) * COLS)
        in_maps.append(
            {
                "xT": xT_b[b],
                "wq": np.ascontiguousarray(WqT[:, cols]),
                "wk": np.ascontiguousarray(WkT[:, cols]),
                "wv": np.ascontiguousarray(WvT[:, cols]),
                "bq": np.ascontiguousarray(bq32[cols].reshape(2, 128).T),
                "bk": np.ascontiguousarray(bk32[cols].reshape(2, 128).T),
                "bv": np.ascontiguousarray(bv32[cols].reshape(2, 128).T),
                "keepT": keepT_b[b],
            }
        )
    return in_maps


def kernel(x, mask, Wq, bq, Wk, bk, Wv, bv, _trace=False):
    from concourse.bass_utils import run_bass_kernel_spmd

    if "nc" not in _cache:
        _cache["nc"] = _build_nc()
    nc = _cache["nc"]

    in_maps = _prep_in_maps(x, mask, Wq, bq, Wk, bk, Wv, bv)
    res = run_bass_kernel_spmd(
        nc, in_maps, core_ids=list(range(N_CORES)), trace=_trace
    )
    _cache["last_result"] = res

    out = np.empty((B, S, D), np.float32)
    for c in range(N_CORES):
        b, g = divmod(c, 4)
        out[b, :, g * COLS : (g + 1) * COLS] = np.asarray(
            res.results[c]["o"], np.float32
        )
    return out



# revision 53
# speedup vs baseline: 1.1253x; 1.1253x over previous
"""Multi-head attention (B=2, S=2048, D=1024, H=16) on 8 Trainium2 cores.

Sharding: core c handles batch b = c//4 and head group g = c%4 (4 heads,
256 of the 1024 QKV output columns). Zero-communication: each core
computes its output slice fully.

Per-core structure (v2 — scheduled for ACT-bound steady state):

  1. Inputs DMA'd in chunks (xT by 512-col chunks, keepT by j-blocks) so
     projections can start ~5us in.
  2. q/k projected in transposed layout qT/kT [dh, s] in 512-col chunks
     (8 K-tiled matmuls each, bias folded into the DVE eviction as a
     per-partition tensor_scalar_add). v projected in natural layout
     [s, dh] per 128-row tile, all 4 heads at once (xT tile stationary),
     with the bias as a rank-1 ones*bv matmul into the same PSUM group.
     Only the first chunks are emitted up front; the rest ride as
     fillers inside the attention loop's PE slack (attention is
     ACT-bound at ~1.1us/iter vs ~0.64us of PE work).
  3. Attention per head-pair in transposed layout: logitsT[sk, sq] =
     kT_tile.T @ qT (two K=64 matmuls row-packed on disjoint PE row
     halves, concurrent), expw = Exp(scale*logits) on ACT, masked by
     multiplying with keepT = (~mask).T in bf16 {0,1} on DVE (exact:
     exp(-1e9) underflows to 0 in fp32).
  4. PV with a ones-augmented V: out_augT[dh+1, sq] += [v|1].T @ expw —
     row 64 accumulates the softmax denominator for free.
  5. Tail per (head, j-block): evict PV PSUM -> bf16 SBUF, transpose
     back to natural [sq, dh] with the DMA xbar (dma_start_transpose,
     frees the PE and most of the old DVE tail), normalize rows by
     1/rowsum on DVE, DMA out.

Matmuls run in bf16 (inputs cast on host), accumulation in fp32 PSUM.
"""

import numpy as np

B, S, D, H = 2, 2048, 1024, 16
HD = D // H  # 64
HEADS_PER_CORE = 4
COLS = HEADS_PER_CORE * HD  # 256
N_CORES = 8
KT = D // 128  # 8 contraction tiles for projections
ST = S // 128  # 16 s tiles
NC = 4  # 512-col chunks of S
SCALE = 1.0 / np.sqrt(np.float32(D))

_cache = {}


def _build_nc():
    import concourse.bass as bass
    import concourse.mybir as mybir
    import concourse.tile as tile

    f32 = mybir.dt.float32
    bf16 = mybir.dt.bfloat16

    nc = bass.Bass(trn_type="TRN2")

    f8 = mybir.dt.float8e4

    xT = nc.dram_tensor("xT", [D, S], bf16, kind="ExternalInput")
    wq = nc.dram_tensor("wq", [D, COLS], bf16, kind="ExternalInput")
    wk = nc.dram_tensor("wk", [D, COLS], bf16, kind="ExternalInput")
    wv = nc.dram_tensor("wv", [D, COLS], bf16, kind="ExternalInput")
    bq = nc.dram_tensor("bq", [128, 2], f32, kind="ExternalInput")
    bk = nc.dram_tensor("bk", [128, 2], f32, kind="ExternalInput")
    bv = nc.dram_tensor("bv", [128, 2], f32, kind="ExternalInput")
    keepT = nc.dram_tensor("keepT", [S, S], bf16, kind="ExternalInput")
    oT = nc.dram_tensor("oT", [HD + 1, 16, 512], bf16, kind="ExternalOutput")

    with tile.TileContext(nc) as tc:
        with (
            tc.tile_pool(name="singles", bufs=1) as singles,
            tc.tile_pool(name="persist", bufs=1) as persist,
            tc.tile_pool(name="big_ps", bufs=2, space="PSUM") as big_ps,
            tc.tile_pool(name="pv_ps", bufs=2, space="PSUM") as pv_ps,
            tc.tile_pool(name="tr_ps", bufs=2, space="PSUM") as tr_ps,
            tc.tile_pool(name="expw", bufs=8) as expw_pool,
            tc.tile_pool(name="expw2", bufs=8) as expw2_pool,
            tc.tile_pool(name="tails", bufs=6) as tails,
            tc.tile_pool(name="vstage", bufs=2) as vstage,
        ):
            # ---- constants / small inputs ----
            ones_col = singles.tile([1, 128], bf16)
            nc.vector.memset(ones_col, 1.0)

            # ---- bulk inputs. HBM is ~0.35 MB/us per core and the total
            # input is ~17 MB (~50us), so ordering decides when compute can
            # start: the first QK needs only wk blk0 + xT chunk0 + wq blk0
            # (1.5 MB), and the first mask-mul needs keepT j0 (2 MB, on the
            # otherwise-idle ACT hwdge queue so its issue doesn't serialize
            # behind the Sync queue).
            wk_sb = persist.tile([128, KT, COLS], bf16)
            wq_sb = persist.tile([128, KT, COLS], bf16)
            wv_sb = persist.tile([128, KT, COLS], bf16)
            xT_sb = persist.tile([128, KT, S], bf16)
            keepT_sb = persist.tile([128, ST, S], bf16)
            bq_sb = singles.tile([128, 2], f32)
            bk_sb = singles.tile([128, 2], f32)
            bv_sb = singles.tile([128, 2], f32)

            xT_r = xT[:, :].rearrange("(kt p) s -> p kt s", p=128)
            keepT_r = keepT[:, :].rearrange("(i p) s -> p i s", p=128)

            def xchunk(c, eng):
                eng.dma_start(
                    out=xT_sb[:, :, c * 512 : (c + 1) * 512],
                    in_=xT_r[:, :, c * 512 : (c + 1) * 512],
                )

            def kchunk(j, eng):
                eng.dma_start(
                    out=keepT_sb[:, :, j * 512 : (j + 1) * 512],
                    in_=keepT_r[:, :, j * 512 : (j + 1) * 512],
                )

            def wblk(w_sb, w_dram, blk, eng):
                eng.dma_start(
                    out=w_sb[:, :, blk * 128 : (blk + 1) * 128],
                    in_=w_dram[:, blk * 128 : (blk + 1) * 128].rearrange(
                        "(kt p) c -> p kt c", p=128
                    ),
                )

            kchunk(0, nc.scalar)  # ACT hwdge queue — idle until the first exp
            # Pre-load the ACT exp table set during the DMA window so the
            # first real ACTIVATE doesn't pay the ~2.7us table load.
            scratch = singles.tile([1, 128], bf16)
            nc.scalar.activation(
                out=scratch,
                in_=ones_col,
                func=mybir.ActivationFunctionType.Exp,
            )
            nc.scalar.dma_start(
                out=wv_sb, in_=wv[:, :].rearrange("(kt p) c -> p kt c", p=128)
            )
            xchunk(2, nc.scalar)
            xchunk(3, nc.scalar)
            nc.sync.dma_start(
                out=wk_sb, in_=wk[:, :].rearrange("(kt p) c -> p kt c", p=128)
            )
            xchunk(0, nc.sync)
            nc.sync.dma_start(
                out=wq_sb, in_=wq[:, :].rearrange("(kt p) c -> p kt c", p=128)
            )
            nc.sync.dma_start(out=bk_sb, in_=bk[:, :])
            nc.sync.dma_start(out=bq_sb, in_=bq[:, :])
            xchunk(1, nc.sync)
            nc.sync.dma_start(out=bv_sb, in_=bv[:, :])
            kchunk(1, nc.sync)
            kchunk(2, nc.sync)
            kchunk(3, nc.sync)

            # ---- projection targets ----
            qT_sb = persist.tile([128, 2, S], bf16)
            kT_sb = persist.tile([128, 2, S], bf16)
            # v is projected in the same transposed layout vT [2 heads x 64,
            # s] (weights stationary: 1.7us per 512-col chunk, vs ~4us for
            # the x-stationary natural-layout form whose per-(kt,st)
            # LDWEIGHTS dominates), then DMA-xbar-transposed per 128-col
            # tile into the natural ones-augmented layout the PV matmul
            # needs: v_aug[p, st, h, 0:64] = v, v_aug[p, st, h, 64] = 1.
            vT_sb = persist.tile([128, 2, S], bf16)
            v_aug = persist.tile([128, ST, HEADS_PER_CORE, HD + 1], bf16)
            nc.vector.memset(v_aug[:, :, :, HD : HD + 1], 1.0)

            # Projections are emitted as small "quarters" (2 matmuls each) so
            # a filler never delays the QK->ACT critical path by more than
            # ~0.45us — one quarter fits the per-iteration PE slack under the
            # ACT-paced steady state. Each chunk is 4 quarters sharing a
            # PSUM tile via closure state; the last quarter evicts.
            def make_qk_quarters(which, blk, c):
                w_sb, b_sb, dst = (
                    (wq_sb, bq_sb, qT_sb),
                    (wk_sb, bk_sb, kT_sb),
                    (wv_sb, bv_sb, vT_sb),
                )[which]
                state = {}

                def quarter(k0):
                    def run():
                        if k0 == 0:
                            state["ps"] = tr_ps.tile(
                                [128, 512], f32, tag="tr", name="qk_ps"
                            )
                        ps = state["ps"]
                        for kt in range(k0, k0 + 2):
                            nc.tensor.matmul(
                                ps,
                                lhsT=w_sb[:, kt, blk * 128 : (blk + 1) * 128],
                                rhs=xT_sb[:, kt, c * 512 : (c + 1) * 512],
                                start=(kt == 0),
                                stop=(kt == KT - 1),
                                skip_group_check=True,
                            )
                        if k0 == 6:
                            nc.vector.tensor_scalar_add(
                                out=dst[:, blk, c * 512 : (c + 1) * 512],
                                in0=ps,
                                scalar1=b_sb[:, blk : blk + 1],
                            )

                    return run

                return [quarter(k0) for k0 in (0, 2, 4, 6)]

            # After a vT chunk is projected, one [128, 512] xbar transpose
            # moves it to natural layout in a contiguous staging tile
            # (3D out [128, 4, 128]: transposed row r = c*128 + p = s
            # within the chunk), then a strided DVE copy interleaves it
            # into v_aug's per-head 65-col groups.
            def make_v_quarters(p, c):
                qs = make_qk_quarters(2, p, c)
                q3 = qs[3]

                def last():
                    q3()
                    vst = vstage.tile([128, 4, 2 * HD], bf16, name="vst")
                    nc.sync.dma_start_transpose(
                        out=vst,
                        in_=vT_sb[:, p, c * 512 : (c + 1) * 512],
                    )
                    nc.vector.tensor_copy(
                        out=v_aug[:, 4 * c : 4 * c + 4, 2 * p : 2 * p + 2, 0:HD],
                        in_=vst.rearrange("p c (h d) -> p c h d", h=2),
                    )

                return [qs[0], qs[1], qs[2], last]

            # ---- attention ----
            # Head pair (2hp, 2hp+1) shares one [128,1024] logits PSUM tile:
            # head e in cols 0-511 (PE rows 0-63), head o in cols 512-1023
            # (PE rows 64-127 via auto tile_position). The two K=64 matmuls
            # run concurrently on disjoint row groups; one wide exp covers
            # both heads.
            def attention_pair(hp, fillers):
                blk = hp
                for j in range(4):  # sq blocks of 512
                    pvs = [
                        pv_ps.tile([HD + 1, 512], f32, tag="pv", name=f"pv{e}")
                        for e in range(2)
                    ]
                    pending_pv = None
                    for i in range(ST):  # sk tiles of 128
                        lgp = big_ps.tile([128, 1024], f32, tag="big")
                        for e in range(2):
                            po = e * 64
                            nc.tensor.matmul(
                                lgp[:, e * 512 : (e + 1) * 512],
                                lhsT=kT_sb[
                                    po : po + 64, blk, i * 128 : (i + 1) * 128
                                ],
                                rhs=qT_sb[
                                    po : po + 64, blk, j * 512 : (j + 1) * 512
                                ],
                                start=True,
                                stop=True,
                                skip_group_check=True,
                            )
                        # PV of the previous iteration is emitted AFTER this
                        # iteration's QK: its mask dependency resolved during
                        # the previous exp, so it never stalls the PE queue
                        # ahead of the QK the ACT critical path waits on.
                        if pending_pv is not None:
                            pending_pv()
                        for f in fillers.get((j, i), ()):
                            f()
                        ex = expw_pool.tile([128, 1024], bf16)
                        nc.scalar.activation(
                            out=ex,
                            in_=lgp,
                            func=mybir.ActivationFunctionType.Exp,
                            scale=float(SCALE),
                        )
                        # mask: multiply both heads' halves by the same keepT
                        # slice, read twice via a stride-0 broadcast dim
                        ex2 = expw2_pool.tile([128, 1024], bf16)
                        k_ap = keepT_sb[:, i, j * 512 : (j + 1) * 512]
                        k_bcast = bass.AP(
                            tensor=k_ap.tensor,
                            offset=k_ap.offset,
                            ap=[k_ap.ap[0], [0, 2], *k_ap.ap[1:]],
                        )
                        nc.vector.tensor_mul(
                            out=ex2.rearrange("p (e n) -> p e n", e=2),
                            in0=ex.rearrange("p (e n) -> p e n", e=2),
                            in1=k_bcast,
                        )

                        def mk_pv(i, ex2):
                            def run():
                                for e in range(2):
                                    nc.tensor.matmul(
                                        pvs[e],
                                        lhsT=v_aug[:, i, 2 * hp + e, :],
                                        rhs=ex2[:, e * 512 : (e + 1) * 512],
                                        start=(i == 0),
                                        stop=(i == ST - 1),
                                        skip_group_check=True,
                                    )

                            return run

                        pending_pv = mk_pv(i, ex2)
                    pending_pv()
                    # tail: evict both heads' unnormalized PV (+denom
                    # row) to bf16 SBUF, DMA straight out; host normalizes.
                    for e in range(2):
                        h = 2 * hp + e
                        pv_sb = tails.tile(
                            [HD + 1, 512], bf16, tag="pvsb", name=f"pv_sb{e}"
                        )
                        nc.vector.tensor_copy(out=pv_sb, in_=pvs[e])
                        nc.sync.dma_start(out=oT[:, h * 4 + j, :], in_=pv_sb)

            # ---- emission order = scheduling priority ----
            Q, K = 0, 1
            # ~16 dummy matmuls on memset data, accumulating into a pv-pool
            # PSUM tile (unused until the first PV at ~26us): the PE's HAM
            # clock gate only reaches 2.4 GHz after ~3.4us of sustained
            # activity and the prologue's DMAs land around t=20us — these
            # warm the array during the dead window so the prologue and
            # early fillers run at full clock.
            warm_sb = singles.tile([128, 512], bf16)
            nc.vector.memset(warm_sb, 0.0)
            warm_ps = pv_ps.tile([HD + 1, 512], f32, tag="pv", name="warm_ps")
            NWARM = 16
            for w in range(NWARM):
                nc.tensor.matmul(
                    warm_ps,
                    lhsT=warm_sb[:, 0 : HD + 1],
                    rhs=warm_sb[:, :],
                    start=(w == 0),
                    stop=(w == NWARM - 1),
                    skip_group_check=True,
                )
            # prologue: only what iteration (0,0) needs — k/q blk0 cols
            # 0-511 and v pair0 chunk0. k and q groups interleaved so each
            # group's DVE eviction overlaps the other group's matmuls.
            k0c0 = make_qk_quarters(K, 0, 0)
            q0c0 = make_qk_quarters(Q, 0, 0)
            for a, b in zip(k0c0, q0c0):
                a()
                b()
            for u in make_v_quarters(0, 0):
                u()

            qk_units = {
                (w, blk, c): make_qk_quarters(w, blk, c)
                for w in (K, Q)
                for blk in (0, 1)
                for c in range(NC)
                if not (blk == 0 and c == 0)
            }
            v_units = {
                (p, c): make_v_quarters(p, c)
                for p in range(2)
                for c in range(NC)
                if not (p == 0 and c == 0)
            }

            # Quarters placed by deadline, at most one (2 matmuls ≈ the PE
            # slack under an ACT-paced iteration) per iteration wherever
            # possible. Hard deadlines: K0/V0 chunk c feed j-block 0 from
            # iteration 4c; Q0 chunk c feeds j-block c; pair-1 units (K1,
            # V1, Q1) are needed from pair-1 iteration 4c / j-block c.
            def place(fillers, j, i0, us):
                for n, u in enumerate(us):
                    idx = i0 + n
                    fillers.setdefault((j + idx // ST, idx % ST), []).append(u)

            fillers0, fillers1 = {}, {}
            # j0: K0/V0 chunks 1-3 must land just ahead of their i-tiles —
            # two quarters per iteration here (PE-heavy window, unavoidable).
            place(fillers0, 0, 0, qk_units[(K, 0, 1)])
            place(fillers0, 0, 0, v_units[(0, 1)])
            place(fillers0, 0, 4, qk_units[(K, 0, 2)])
            place(fillers0, 0, 4, v_units[(0, 2)])
            place(fillers0, 0, 8, qk_units[(K, 0, 3)])
            place(fillers0, 0, 8, v_units[(0, 3)])
            place(fillers0, 0, 12, qk_units[(Q, 0, 1)])
            # j1..j3: one quarter per iteration.
            place(fillers0, 1, 0, qk_units[(K, 1, 0)])
            place(fillers0, 1, 4, v_units[(1, 0)])
            place(fillers0, 1, 8, qk_units[(Q, 0, 2)])
            place(fillers0, 1, 12, qk_units[(K, 1, 1)])
            place(fillers0, 2, 0, v_units[(1, 1)])
            place(fillers0, 2, 4, qk_units[(K, 1, 2)])
            place(fillers0, 2, 8, qk_units[(Q, 0, 3)])
            place(fillers0, 2, 12, v_units[(1, 2)])
            place(fillers0, 3, 0, qk_units[(K, 1, 3)])
            place(fillers0, 3, 4, v_units[(1, 3)])
            place(fillers0, 3, 8, qk_units[(Q, 1, 0)])
            place(fillers1, 0, 0, qk_units[(Q, 1, 1)])
            place(fillers1, 1, 0, qk_units[(Q, 1, 2)])
            place(fillers1, 2, 0, qk_units[(Q, 1, 3)])
            attention_pair(0, fillers0)
            attention_pair(1, fillers1)

    # Workaround: this container's walrus encodes at most one sync wait per
    # instruction — split multi-wait instructions into single-wait NoOps.
    _split_multiwait(nc)
    return nc


def _split_multiwait(nc, max_waits: int = 1):
    import concourse.mybir as mybir

    for f in nc.m.functions:
        for blk in f.blocks:
            out = []
            changed = False
            for inst in blk.instructions:
                si = inst.sync_info
                if si is not None and len(si.on_wait) > max_waits:
                    waits = list(si.on_wait)
                    extra = waits[: len(waits) - max_waits]
                    keep = waits[len(waits) - max_waits :]
                    for k, w in enumerate(extra):
                        out.append(
                            mybir.InstNoOp(
                                name=f"{inst.name}-wfx{k}",
                                engine=inst.engine,
                                sync_info=mybir.SyncInfo(on_wait=[w], on_update=[]),
                                bass_nofuse=True,
                            )
                        )
                    inst.sync_info = mybir.SyncInfo(
                        on_wait=keep, on_update=list(si.on_update)
                    )
                    changed = True
                out.append(inst)
            if changed:
                blk.instructions = out


def _prep_in_maps(x, mask, Wq, bq, Wk, bk, Wv, bv):
    import ml_dtypes

    bf16 = ml_dtypes.bfloat16
    x = np.asarray(x, np.float32)
    mask = np.asarray(mask, bool)

    xT_b = [np.ascontiguousarray(x[b].T).astype(bf16) for b in range(B)]
    keepT_b = [
        np.ascontiguousarray((~mask[b, 0]).T).astype(bf16) for b in range(B)
    ]
    WqT = np.asarray(Wq, np.float32).T.astype(bf16)
    WkT = np.asarray(Wk, np.float32).T.astype(bf16)
    WvT = np.asarray(Wv, np.float32).T.astype(bf16)
    bq32 = np.asarray(bq, np.float32)
    bk32 = np.asarray(bk, np.float32)
    bv32 = np.asarray(bv, np.float32)

    in_maps = []
    for c in range(N_CORES):
        b, g = divmod(c, 4)
        cols = slice(g * COLS, (g + 1) * COLS)
        in_maps.append(
            {
                "xT": xT_b[b],
                "wq": np.ascontiguousarray(WqT[:, cols]),
                "wk": np.ascontiguousarray(WkT[:, cols]),
                "wv": np.ascontiguousarray(WvT[:, cols]),
                "bq": np.ascontiguousarray(bq32[cols].reshape(2, 128).T),
                "bk": np.ascontiguousarray(bk32[cols].reshape(2, 128).T),
                "bv": np.ascontiguousarray(bv32[cols].reshape(2, 128).T),
                "keepT": keepT_b[b],
            }
        )
    return in_maps


def kernel(x, mask, Wq, bq, Wk, bk, Wv, bv, _trace=False):
    from concourse.bass_utils import run_bass_kernel_spmd

    if "nc" not in _cache:
        _cache["nc"] = _build_nc()
    nc = _cache["nc"]

    in_maps = _prep_in_maps(x, mask, Wq, bq, Wk, bk, Wv, bv)
    res = run_bass_kernel_spmd(
        nc, in_maps, core_ids=list(range(N_CORES)), trace=_trace
    )
    _cache["last_result"] = res

    out = np.empty((B, S, D), np.float32)
    for c in range(N_CORES):
        b, g = divmod(c, 4)
        arr = np.asarray(res.results[c]["oT"], np.float32)
        blocks = arr[0:HD] / arr[HD]
        v = blocks.reshape(HD, 4, 4, 512).transpose(2, 3, 1, 0)
        out[b, :, g * COLS : (g + 1) * COLS] = v.reshape(S, COLS)
    return out



# revision 54
# speedup vs baseline: 1.1822x; 1.0506x over previous
"""Multi-head attention (B=2, S=2048, D=1024, H=16) on 8 Trainium2 cores.

Sharding: core c handles batch b = c//4 and head group g = c%4 (4 heads,
256 of the 1024 QKV output columns). Zero-communication: each core
computes its output slice fully.

Per-core structure (v2 — scheduled for ACT-bound steady state):

  1. Inputs DMA'd in chunks (xT by 512-col chunks, keepT by j-blocks) so
     projections can start ~5us in.
  2. q/k projected in transposed layout qT/kT [dh, s] in 512-col chunks
     (8 K-tiled matmuls each, bias folded into the DVE eviction as a
     per-partition tensor_scalar_add). v projected in natural layout
     [s, dh] per 128-row tile, all 4 heads at once (xT tile stationary),
     with the bias as a rank-1 ones*bv matmul into the same PSUM group.
     Only the first chunks are emitted up front; the rest ride as
     fillers inside the attention loop's PE slack (attention is
     ACT-bound at ~1.1us/iter vs ~0.64us of PE work).
  3. Attention per head-pair in transposed layout: logitsT[sk, sq] =
     kT_tile.T @ qT (two K=64 matmuls row-packed on disjoint PE row
     halves, concurrent), expw = Exp(scale*logits) on ACT, masked by
     multiplying with keepT = (~mask).T in bf16 {0,1} on DVE (exact:
     exp(-1e9) underflows to 0 in fp32).
  4. PV with a ones-augmented V: out_augT[dh+1, sq] += [v|1].T @ expw —
     row 64 accumulates the softmax denominator for free.
  5. Tail per (head, j-block): evict PV PSUM -> bf16 SBUF, transpose
     back to natural [sq, dh] with the DMA xbar (dma_start_transpose,
     frees the PE and most of the old DVE tail), normalize rows by
     1/rowsum on DVE, DMA out.

Matmuls run in bf16 (inputs cast on host), accumulation in fp32 PSUM.
"""

import numpy as np

B, S, D, H = 2, 2048, 1024, 16
HD = D // H  # 64
HEADS_PER_CORE = 4
COLS = HEADS_PER_CORE * HD  # 256
N_CORES = 8
KT = D // 128  # 8 contraction tiles for projections
ST = S // 128  # 16 s tiles
NC = 4  # 512-col chunks of S
SCALE = 1.0 / np.sqrt(np.float32(D))

_cache = {}


def _build_nc():
    import concourse.bass as bass
    import concourse.mybir as mybir
    import concourse.tile as tile

    f32 = mybir.dt.float32
    bf16 = mybir.dt.bfloat16

    nc = bass.Bass(trn_type="TRN2")

    f8 = mybir.dt.float8e4

    xT = nc.dram_tensor("xT", [D, S], bf16, kind="ExternalInput")
    wq = nc.dram_tensor("wq", [D, COLS], bf16, kind="ExternalInput")
    wk = nc.dram_tensor("wk", [D, COLS], bf16, kind="ExternalInput")
    wv = nc.dram_tensor("wv", [D, COLS], bf16, kind="ExternalInput")
    bq = nc.dram_tensor("bq", [128, 2], f32, kind="ExternalInput")
    bk = nc.dram_tensor("bk", [128, 2], f32, kind="ExternalInput")
    bv = nc.dram_tensor("bv", [128, 2], f32, kind="ExternalInput")
    keepT = nc.dram_tensor("keepT", [S, S], bf16, kind="ExternalInput")
    oT = nc.dram_tensor("oT", [HD + 1, 16, 512], bf16, kind="ExternalOutput")

    with tile.TileContext(nc) as tc:
        with (
            tc.tile_pool(name="singles", bufs=1) as singles,
            tc.tile_pool(name="persist", bufs=1) as persist,
            tc.tile_pool(name="big_ps", bufs=2, space="PSUM") as big_ps,
            tc.tile_pool(name="pv_ps", bufs=2, space="PSUM") as pv_ps,
            tc.tile_pool(name="tr_ps", bufs=2, space="PSUM") as tr_ps,
            tc.tile_pool(name="expw", bufs=8) as expw_pool,
            tc.tile_pool(name="expw2", bufs=8) as expw2_pool,
            tc.tile_pool(name="tails", bufs=6) as tails,
            tc.tile_pool(name="vstage", bufs=2) as vstage,
        ):
            # ---- constants / small inputs ----
            ones_col = singles.tile([1, 128], bf16)
            nc.vector.memset(ones_col, 1.0)

            # ---- bulk inputs. HBM is ~0.35 MB/us per core and the total
            # input is ~17 MB (~50us), so ordering decides when compute can
            # start: the first QK needs only wk blk0 + xT chunk0 + wq blk0
            # (1.5 MB), and the first mask-mul needs keepT j0 (2 MB, on the
            # otherwise-idle ACT hwdge queue so its issue doesn't serialize
            # behind the Sync queue).
            wk_sb = persist.tile([128, KT, COLS], bf16)
            wq_sb = persist.tile([128, KT, COLS], bf16)
            wv_sb = persist.tile([128, KT, COLS], bf16)
            xT_sb = persist.tile([128, KT, S], bf16)
            keepT_sb = persist.tile([128, ST, S], bf16)
            bq_sb = singles.tile([128, 2], f32)
            bk_sb = singles.tile([128, 2], f32)
            bv_sb = singles.tile([128, 2], f32)

            xT_r = xT[:, :].rearrange("(kt p) s -> p kt s", p=128)
            keepT_r = keepT[:, :].rearrange("(i p) s -> p i s", p=128)

            def xchunk(c, eng):
                eng.dma_start(
                    out=xT_sb[:, :, c * 512 : (c + 1) * 512],
                    in_=xT_r[:, :, c * 512 : (c + 1) * 512],
                )

            def kchunk(j, eng):
                eng.dma_start(
                    out=keepT_sb[:, :, j * 512 : (j + 1) * 512],
                    in_=keepT_r[:, :, j * 512 : (j + 1) * 512],
                )

            def wblk(w_sb, w_dram, blk, eng):
                eng.dma_start(
                    out=w_sb[:, :, blk * 128 : (blk + 1) * 128],
                    in_=w_dram[:, blk * 128 : (blk + 1) * 128].rearrange(
                        "(kt p) c -> p kt c", p=128
                    ),
                )

            kchunk(0, nc.scalar)  # ACT hwdge queue — idle until the first exp
            # Pre-load the ACT exp table set during the DMA window so the
            # first real ACTIVATE doesn't pay the ~2.7us table load.
            scratch = singles.tile([1, 128], bf16)
            nc.scalar.activation(
                out=scratch,
                in_=ones_col,
                func=mybir.ActivationFunctionType.Exp,
            )
            nc.scalar.dma_start(
                out=wv_sb, in_=wv[:, :].rearrange("(kt p) c -> p kt c", p=128)
            )
            xchunk(2, nc.scalar)
            xchunk(3, nc.scalar)
            nc.sync.dma_start(
                out=wk_sb, in_=wk[:, :].rearrange("(kt p) c -> p kt c", p=128)
            )
            xchunk(0, nc.sync)
            nc.sync.dma_start(
                out=wq_sb, in_=wq[:, :].rearrange("(kt p) c -> p kt c", p=128)
            )
            nc.sync.dma_start(out=bk_sb, in_=bk[:, :])
            nc.sync.dma_start(out=bq_sb, in_=bq[:, :])
            xchunk(1, nc.sync)
            nc.sync.dma_start(out=bv_sb, in_=bv[:, :])
            kchunk(1, nc.sync)
            kchunk(2, nc.sync)
            kchunk(3, nc.sync)

            # ---- projection targets ----
            qT_sb = persist.tile([128, 2, S], bf16)
            kT_sb = persist.tile([128, 2, S], bf16)
            # v is projected in the same transposed layout vT [2 heads x 64,
            # s] (weights stationary: 1.7us per 512-col chunk, vs ~4us for
            # the x-stationary natural-layout form whose per-(kt,st)
            # LDWEIGHTS dominates), then DMA-xbar-transposed per 128-col
            # tile into the natural ones-augmented layout the PV matmul
            # needs: v_aug[p, st, h, 0:64] = v, v_aug[p, st, h, 64] = 1.
            vT_sb = persist.tile([128, 2, S], bf16)
            v_aug = persist.tile([128, ST, HEADS_PER_CORE, HD + 1], bf16)
            nc.vector.memset(v_aug[:, :, :, HD : HD + 1], 1.0)

            # Projections are emitted as small "quarters" (2 matmuls each) so
            # a filler never delays the QK->ACT critical path by more than
            # ~0.45us — one quarter fits the per-iteration PE slack under the
            # ACT-paced steady state. Each chunk is 4 quarters sharing a
            # PSUM tile via closure state; the last quarter evicts.
            def make_qk_quarters(which, blk, c):
                w_sb, b_sb, dst = (
                    (wq_sb, bq_sb, qT_sb),
                    (wk_sb, bk_sb, kT_sb),
                    (wv_sb, bv_sb, vT_sb),
                )[which]
                state = {}

                def quarter(k0):
                    def run():
                        if k0 == 0:
                            state["ps"] = tr_ps.tile(
                                [128, 512], f32, tag="tr", name="qk_ps"
                            )
                        ps = state["ps"]
                        for kt in range(k0, k0 + 2):
                            nc.tensor.matmul(
                                ps,
                                lhsT=w_sb[:, kt, blk * 128 : (blk + 1) * 128],
                                rhs=xT_sb[:, kt, c * 512 : (c + 1) * 512],
                                start=(kt == 0),
                                stop=(kt == KT - 1),
                                skip_group_check=True,
                            )
                        if k0 == 6:
                            # first half only — the second half is its own
                            # closure so the two ~460ns DVE passes land on
                            # different iterations (one 751ns pass + the
                            # 688ns mask would exceed the ACT-paced iter).
                            nc.vector.tensor_scalar_add(
                                out=dst[:, blk, c * 512 : c * 512 + 256],
                                in0=ps[:, 0:256],
                                scalar1=b_sb[:, blk : blk + 1],
                            )

                    return run

                def evict_b():
                    nc.vector.tensor_scalar_add(
                        out=dst[:, blk, c * 512 + 256 : (c + 1) * 512],
                        in0=state["ps"][:, 256:512],
                        scalar1=b_sb[:, blk : blk + 1],
                    )

                return [quarter(0), quarter(2), quarter(4), quarter(6), evict_b]

            # After a vT chunk is projected, one [128, 512] xbar transpose
            # moves it to natural layout in a contiguous staging tile
            # (3D out [128, 4, 128]: transposed row r = c*128 + p = s
            # within the chunk), then a strided DVE copy interleaves it
            # into v_aug's per-head 65-col groups.
            def make_v_quarters(p, c):
                qs = make_qk_quarters(2, p, c)
                q4 = qs[4]

                def last():
                    q4()
                    vst = vstage.tile([128, 4, 2 * HD], bf16, name="vst")
                    nc.sync.dma_start_transpose(
                        out=vst,
                        in_=vT_sb[:, p, c * 512 : (c + 1) * 512],
                    )
                    nc.vector.tensor_copy(
                        out=v_aug[:, 4 * c : 4 * c + 4, 2 * p : 2 * p + 2, 0:HD],
                        in_=vst.rearrange("p c (h d) -> p c h d", h=2),
                    )

                return [qs[0], qs[1], qs[2], qs[3], last]

            # ---- attention ----
            # Head pair (2hp, 2hp+1) shares one [128,1024] logits PSUM tile:
            # head e in cols 0-511 (PE rows 0-63), head o in cols 512-1023
            # (PE rows 64-127 via auto tile_position). The two K=64 matmuls
            # run concurrently on disjoint row groups; one wide exp covers
            # both heads.
            def attention_pair(hp, fillers):
                blk = hp
                for j in range(4):  # sq blocks of 512
                    pvs = [
                        pv_ps.tile([HD + 1, 512], f32, tag="pv", name=f"pv{e}")
                        for e in range(2)
                    ]
                    pending_pv = None
                    for i in range(ST):  # sk tiles of 128
                        lgp = big_ps.tile([128, 1024], f32, tag="big")
                        for e in range(2):
                            po = e * 64
                            nc.tensor.matmul(
                                lgp[:, e * 512 : (e + 1) * 512],
                                lhsT=kT_sb[
                                    po : po + 64, blk, i * 128 : (i + 1) * 128
                                ],
                                rhs=qT_sb[
                                    po : po + 64, blk, j * 512 : (j + 1) * 512
                                ],
                                start=True,
                                stop=True,
                                skip_group_check=True,
                            )
                        # PV of the previous iteration is emitted AFTER this
                        # iteration's QK: its mask dependency resolved during
                        # the previous exp, so it never stalls the PE queue
                        # ahead of the QK the ACT critical path waits on.
                        if pending_pv is not None:
                            pending_pv()
                        for f in fillers.get((j, i), ()):
                            f()
                        ex = expw_pool.tile([128, 1024], bf16)
                        nc.scalar.activation(
                            out=ex,
                            in_=lgp,
                            func=mybir.ActivationFunctionType.Exp,
                            scale=float(SCALE),
                        )
                        # mask: multiply both heads' halves by the same keepT
                        # slice, read twice via a stride-0 broadcast dim
                        ex2 = expw2_pool.tile([128, 1024], bf16)
                        k_ap = keepT_sb[:, i, j * 512 : (j + 1) * 512]
                        k_bcast = bass.AP(
                            tensor=k_ap.tensor,
                            offset=k_ap.offset,
                            ap=[k_ap.ap[0], [0, 2], *k_ap.ap[1:]],
                        )
                        nc.vector.tensor_mul(
                            out=ex2.rearrange("p (e n) -> p e n", e=2),
                            in0=ex.rearrange("p (e n) -> p e n", e=2),
                            in1=k_bcast,
                        )

                        def mk_pv(i, ex2):
                            def run():
                                for e in range(2):
                                    nc.tensor.matmul(
                                        pvs[e],
                                        lhsT=v_aug[:, i, 2 * hp + e, :],
                                        rhs=ex2[:, e * 512 : (e + 1) * 512],
                                        start=(i == 0),
                                        stop=(i == ST - 1),
                                        skip_group_check=True,
                                    )

                            return run

                        pending_pv = mk_pv(i, ex2)
                    pending_pv()
                    # tail: evict both heads' unnormalized PV (+denom
                    # row) to bf16 SBUF, DMA straight out; host normalizes.
                    for e in range(2):
                        h = 2 * hp + e
                        pv_sb = tails.tile(
                            [HD + 1, 512], bf16, tag="pvsb", name=f"pv_sb{e}"
                        )
                        nc.vector.tensor_copy(out=pv_sb, in_=pvs[e])
                        nc.sync.dma_start(out=oT[:, h * 4 + j, :], in_=pv_sb)

            # ---- emission order = scheduling priority ----
            Q, K = 0, 1
            # ~16 dummy matmuls on memset data, accumulating into a pv-pool
            # PSUM tile (unused until the first PV at ~26us): the PE's HAM
            # clock gate only reaches 2.4 GHz after ~3.4us of sustained
            # activity and the prologue's DMAs land around t=20us — these
            # warm the array during the dead window so the prologue and
            # early fillers run at full clock.
            warm_sb = singles.tile([128, 512], bf16)
            nc.vector.memset(warm_sb, 0.0)
            warm_ps = pv_ps.tile([HD + 1, 512], f32, tag="pv", name="warm_ps")
            NWARM = 16
            for w in range(NWARM):
                nc.tensor.matmul(
                    warm_ps,
                    lhsT=warm_sb[:, 0 : HD + 1],
                    rhs=warm_sb[:, :],
                    start=(w == 0),
                    stop=(w == NWARM - 1),
                    skip_group_check=True,
                )
            # prologue: only what iteration (0,0) needs — k/q blk0 cols
            # 0-511 and v pair0 chunk0. k and q groups interleaved so each
            # group's DVE eviction overlaps the other group's matmuls.
            k0c0 = make_qk_quarters(K, 0, 0)
            q0c0 = make_qk_quarters(Q, 0, 0)
            for a, b in zip(k0c0, q0c0):
                a()
                b()
            for u in make_v_quarters(0, 0):
                u()

            qk_units = {
                (w, blk, c): make_qk_quarters(w, blk, c)
                for w in (K, Q)
                for blk in (0, 1)
                for c in range(NC)
                if not (blk == 0 and c == 0)
            }
            v_units = {
                (p, c): make_v_quarters(p, c)
                for p in range(2)
                for c in range(NC)
                if not (p == 0 and c == 0)
            }

            # Quarters placed by deadline, at most one (2 matmuls ≈ the PE
            # slack under an ACT-paced iteration) per iteration wherever
            # possible. Hard deadlines: K0/V0 chunk c feed j-block 0 from
            # iteration 4c; Q0 chunk c feeds j-block c; pair-1 units (K1,
            # V1, Q1) are needed from pair-1 iteration 4c / j-block c.
            def place(fillers, j, i0, us):
                for n, u in enumerate(us):
                    idx = i0 + n
                    fillers.setdefault((j + idx // ST, idx % ST), []).append(u)

            fillers0, fillers1 = {}, {}
            # j0: K0/V0 chunks 1-3 must land just ahead of their i-tiles —
            # two quarters per iteration here (PE-heavy window, unavoidable).
            place(fillers0, 0, 0, qk_units[(K, 0, 1)])
            place(fillers0, 0, 0, v_units[(0, 1)])
            place(fillers0, 0, 4, qk_units[(K, 0, 2)])
            place(fillers0, 0, 4, v_units[(0, 2)])
            place(fillers0, 0, 8, qk_units[(K, 0, 3)])
            place(fillers0, 0, 8, v_units[(0, 3)])
            # Q0c1 starts at (0,11) so its 5th closure (second eviction
            # half) is emitted at (0,15) — before QK(1,0) reads qT cols
            # 768:1024. At (0,12) it would wrap to (1,0), emitted AFTER
            # that QK (fillers follow QK in the loop) -> fresh-run race.
            place(fillers0, 0, 11, qk_units[(Q, 0, 1)])
            # j1..j3: one quarter per iteration.
            place(fillers0, 1, 0, qk_units[(K, 1, 0)])
            place(fillers0, 1, 4, v_units[(1, 0)])
            place(fillers0, 1, 8, qk_units[(Q, 0, 2)])
            place(fillers0, 1, 12, qk_units[(K, 1, 1)])
            place(fillers0, 2, 0, v_units[(1, 1)])
            place(fillers0, 2, 4, qk_units[(K, 1, 2)])
            place(fillers0, 2, 8, qk_units[(Q, 0, 3)])
            place(fillers0, 2, 12, v_units[(1, 2)])
            place(fillers0, 3, 0, qk_units[(K, 1, 3)])
            place(fillers0, 3, 4, v_units[(1, 3)])
            place(fillers0, 3, 8, qk_units[(Q, 1, 0)])
            place(fillers1, 0, 0, qk_units[(Q, 1, 1)])
            place(fillers1, 1, 0, qk_units[(Q, 1, 2)])
            place(fillers1, 2, 0, qk_units[(Q, 1, 3)])
            attention_pair(0, fillers0)
            attention_pair(1, fillers1)

    # Workaround: this container's walrus encodes at most one sync wait per
    # instruction — split multi-wait instructions into single-wait NoOps.
    _split_multiwait(nc)
    return nc


def _split_multiwait(nc, max_waits: int = 1):
    import concourse.mybir as mybir

    for f in nc.m.functions:
        for blk in f.blocks:
            out = []
            changed = False
            for inst in blk.instructions:
                si = inst.sync_info
                if si is not None and len(si.on_wait) > max_waits:
                    waits = list(si.on_wait)
                    extra = waits[: len(waits) - max_waits]
                    keep = waits[len(waits) - max_waits :]
                    for k, w in enumerate(extra):
                        out.append(
                            mybir.InstNoOp(
                                name=f"{inst.name}-wfx{k}",
                                engine=inst.engine,
                                sync_info=mybir.SyncInfo(on_wait=[w], on_update=[]),
                                bass_nofuse=True,
                            )
                        )
                    inst.sync_info = mybir.SyncInfo(
                        on_wait=keep, on_update=list(si.on_update)
                    )
                    changed = True
                out.append(inst)
            if changed:
                blk.instructions = out


def _prep_in_maps(x, mask, Wq, bq, Wk, bk, Wv, bv):
    import ml_dtypes

    bf16 = ml_dtypes.bfloat16
    x = np.asarray(x, np.float32)
    mask = np.asarray(mask, bool)

    xT_b = [np.ascontiguousarray(x[b].T).astype(bf16) for b in range(B)]
    keepT_b = [
        np.ascontiguousarray((~mask[b, 0]).T).astype(bf16) for b in range(B)
    ]
    WqT = np.asarray(Wq, np.float32).T.astype(bf16)
    WkT = np.asarray(Wk, np.float32).T.astype(bf16)
    WvT = np.asarray(Wv, np.float32).T.astype(bf16)
    bq32 = np.asarray(bq, np.float32)
    bk32 = np.asarray(bk, np.float32)
    bv32 = np.asarray(bv, np.float32)

    in_maps = []
    for c in range(N_CORES):
        b, g = divmod(c, 4)
        cols = slice(g * COLS, (g + 1) * COLS)
        in_maps.append(
            {
                "xT": xT_b[b],
                "wq": np.ascontiguousarray(WqT[:, cols]),
                "wk": np.ascontiguousarray(WkT[:, cols]),
                "wv": np.ascontiguousarray(WvT[:, cols]),
                "bq": np.ascontiguousarray(bq32[cols].reshape(2, 128).T),
                "bk": np.ascontiguousarray(bk32[cols].reshape(2, 128).T),
                "bv": np.ascontiguousarray(bv32[cols].reshape(2, 128).T),
                "keepT": keepT_b[b],
            }
        )
    return in_maps


def kernel(x, mask, Wq, bq, Wk, bk, Wv, bv, _trace=False):
    from concourse.bass_utils import run_bass_kernel_spmd

    if "nc" not in _cache:
        _cache["nc"] = _build_nc()
    nc = _cache["nc"]

    in_maps = _prep_in_maps(x, mask, Wq, bq, Wk, bk, Wv, bv)
    res = run_bass_kernel_spmd(
        nc, in_maps, core_ids=list(range(N_CORES)), trace=_trace
    )
    _cache["last_result"] = res

    out = np.empty((B, S, D), np.float32)
    for c in range(N_CORES):
        b, g = divmod(c, 4)
        arr = np.asarray(res.results[c]["oT"], np.float32)
        blocks = arr[0:HD] / arr[HD]
        v = blocks.reshape(HD, 4, 4, 512).transpose(2, 3, 1, 0)
        out[b, :, g * COLS : (g + 1) * COLS] = v.reshape(S, COLS)
    return out

